# revision 40
# baseline (speedup 1.0000x reference)
"""Trainium2 Bass kernel for BatchWiseTripletDistanceLoss (banded scatter v8).

loss = mean_t relu(cos_d(s[a],s[p]) - cos_d(s[a],s[n]) + margin)
     = mean_t relu(sim[a,n] - sim[a,p] + margin)        (the "1-"s cancel)

Each of 8 cores owns 128 anchor rows (a mod 4) and half the negatives
(n >> 8).  On device it computes u = cos(anchor, all 512 samples) + 2 in
f16 (fp8 DoubleRow matmuls for dot products and squared norms, one
Abs_reciprocal_sqrt table op, an outer product of reciprocal norms), then
one gpsimd local_scatter distributes the u row into
  [palette | banded grid]:
  - palette slot s        <- u at the row's s-th distinct positive column
  - band cells            <- u at each triplet's negative column, with each
    row's triplets packed by positive-slot into width-32 bands (one slot
    per (row, band)).
A second tiny scatter builds cband[row, band] = u_pos of that band's slot.
One fused DVE op computes acc1 = sum max(cband - margin, grid) and another
acc2 = sum cband; the host combines
    sum relu = acc1 - 32*acc2 + 32*margin*n_used_bands
(empty cells/bands cancel exactly since u >= 1 > 0 and the margin is
applied in f32 inside the ALU, so no f16 grid-alignment bias).

The sample columns are processed in two groups (A = permuted cols 0..255,
which include all 128 anchors; B = cols 256..511) with separate DMAs, so
group A's norm/normalize chain overlaps group B's DMA + compute.  Host
work is layout / integer metadata only, plus the final tiny partial-sum
reduction.
"""
import sys

sys.path.insert(0, "/opt/trn_rl_repo")

from contextlib import ExitStack

import numpy as np

import concourse.bacc as bacc
import concourse.tile as tile
from concourse import mybir
from concourse.bass_utils import run_bass_kernel_spmd

DT = mybir.dt
OP = mybir.AluOpType
ACTF = mybir.ActivationFunctionType

N = 512
D = 256
MARGIN = 0.15
NCORES = 8
NROW = 128
WBAND = 32  # band width (cells per band)
HC = 256  # columns per group


def _build_program(s_pad: int, nband: int, m_span: int):
    """Build + compile the SPMD program (identical for all 8 cores)."""
    nc = bacc.Bacc(
        "TRN2", target_bir_lowering=False, debug=False, num_devices=NCORES
    )
    f32, i16, f16, f8 = DT.float32, DT.int16, DT.float16, DT.float8e4

    WIDE = s_pad + nband * WBAND
    NI3 = s_pad * m_span
    # group A: [j, c<256] DR layout (512 cols); group B: same for c>=256,
    # followed by the anchor lhsT block (256 cols).
    d_packa1 = nc.dram_tensor("packa1", [128, 512], f8, kind="ExternalInput").ap()
    d_packa2 = nc.dram_tensor("packa2", [128, 768], f8, kind="ExternalInput").ap()
    d_packb = nc.dram_tensor("packb", [NROW, N + NI3], i16, kind="ExternalInput").ap()
    d_out = nc.dram_tensor("out", [NROW, 2], f32, kind="ExternalOutput").ap()

    with tile.TileContext(nc) as tc, ExitStack() as ctx:
        cpool = ctx.enter_context(tc.tile_pool(name="const", bufs=1))
        wpool = ctx.enter_context(tc.tile_pool(name="work", bufs=1))
        ppool = ctx.enter_context(tc.tile_pool(name="psum", bufs=1, space="PSUM"))

        # ---- input DMAs (three queues) ----------------------------------
        # descriptor writes spread across engines: scalar stays free for the
        # ACT table loads, which otherwise gate the first Square op
        packa1 = cpool.tile([128, 512], f8)
        nc.sync.dma_start(packa1[:], d_packa1)
        packa2 = cpool.tile([128, 768], f8)
        nc.gpsimd.dma_start(packa2[:], d_packa2)
        packb = cpool.tile([NROW, N + NI3], i16)
        nc.sync.dma_start(packb[:], d_packb)
        idxs_all = packb[:, 0:N]
        idxs3 = packb[:, N : N + NI3]
        packl = packa2[:, 512:768]

        # DR lhsT needs 16B-aligned j-step: ones read at [p, 0] and [p, 16]
        ones2 = cpool.tile([128, 32], f8)
        nc.vector.memset(ones2[:], 1.0)
        ones_lhsT = ones2[:].rearrange("p (j x) -> p j x", j=2)[:, :, 0:1]

        # ---- warmups hidden under the DMA phase -------------------------
        # rsqrt ACT table preload (the only ACT table this kernel uses)
        dumin = cpool.tile([1, 1], f32)
        nc.vector.memset(dumin[:], 4.0)
        dumout = cpool.tile([1, 1], f32)
        nc.scalar.activation(dumout[:], dumin[:], ACTF.Abs_reciprocal_sqrt)
        # gpsimd local_scatter ucode IRAM load (~6us, hidden here)
        dmy_d = cpool.tile([128, 2], f16)
        nc.vector.memset(dmy_d[:], 0.0)
        dmy_i = cpool.tile([128, 2], i16)
        nc.vector.memset(dmy_i[:], -1)
        dmy_o = cpool.tile([128, 2], f16)
        nc.gpsimd.local_scatter(
            dmy_o[:], dmy_d[:], dmy_i[:], channels=128, num_elems=2, num_idxs=2
        )

        DR = mybir.MatmulPerfMode.DoubleRow
        grpA = packa1[:].rearrange("p (j c) -> p j c", j=2)
        grpB = packa2[:, 0:512].rearrange("p (j c) -> p j c", j=2)
        lhsT = packl.rearrange("p (j q) -> p j q", j=2)

        u16 = wpool.tile([128, N], f16, tag="u16")
        simps, rbps = [], []
        for g, grp in enumerate((grpA, grpB)):
            # squares (ACT for A, DVE for B -> they run in parallel)
            sqg = wpool.tile([128, 512], f8, tag=f"sq{g}")
            if g == 0:
                nc.scalar.activation(sqg[:], packa1[:], ACTF.Square)
            else:
                nc.vector.tensor_tensor(
                    sqg[:], packa2[:, 0:512], packa2[:, 0:512], OP.mult
                )
            n2g = ppool.tile([1, HC], f32, tag=f"n2{g}")
            nc.tensor.matmul(
                n2g[:],
                ones_lhsT,
                sqg[:].rearrange("p (j c) -> p j c", j=2),
                start=True,
                stop=True,
                perf_mode=DR,
            )
            rrg = wpool.tile([1, HC], f16, tag=f"rr{g}")
            nc.scalar.activation(rrg[:], n2g[:], ACTF.Abs_reciprocal_sqrt)
            rbps.append(rrg)
            simpg = ppool.tile([128, HC], f32, tag=f"simp{g}")
            nc.tensor.matmul(simpg[:], lhsT, grp, start=True, stop=True, perf_mode=DR)
            simps.append(simpg)

        rrA = rbps[0]
        for g in range(2):
            # outer product of reciprocal norms (anchors all in group A)
            rbp2g = ppool.tile([128, HC], f32, tag=f"rbp2{g}")
            nc.tensor.matmul(
                rbp2g[:], rrA[:, 0:128], rbps[g][:], start=True, stop=True
            )
            simp_sbg = wpool.tile([128, HC], f32, tag=f"simp_sb{g}")
            nc.vector.tensor_scalar_add(simp_sbg[:], simps[g][:], 0.0)
            t1g = wpool.tile([128, HC], f32, tag=f"t1{g}")
            nc.vector.tensor_tensor(t1g[:], simp_sbg[:], rbp2g[:], OP.mult)
            nc.vector.tensor_scalar_add(u16[:, g * HC : (g + 1) * HC], t1g[:], 2.0)

        # ---- merged scatter: [palette | banded grid] --------------------
        dst = wpool.tile([NROW, WIDE], f16, tag="dst")
        nc.gpsimd.local_scatter(
            dst[:], u16[:], idxs_all, channels=128, num_elems=WIDE, num_idxs=N
        )
        palv = dst[:, 0:s_pad]
        grid = dst[:, s_pad:WIDE]

        # crep = palette values replicated per span slot (exact f16 copy)
        crep = wpool.tile([NROW, NI3], f16, tag="crep")
        nc.vector.tensor_scalar_add(
            crep[:].rearrange("p (s j) -> p s j", s=s_pad),
            palv.unsqueeze(2).to_broadcast((NROW, s_pad, m_span)),
            0.0,
        )
        cband = wpool.tile([NROW, nband], f16, tag="cband")
        nc.gpsimd.local_scatter(
            cband[:], crep[:], idxs3, channels=128, num_elems=nband, num_idxs=NI3
        )

        # ---- fused evaluate + accumulate --------------------------------
        accs = wpool.tile([NROW, 2], f32, tag="accs")
        scr2 = wpool.tile([NROW, nband], f32, tag="scr2")
        nc.vector.tensor_scalar(
            scr2[:], cband[:], 0.0, 0.0, OP.add, OP.add, accum_out=accs[:, 1:2]
        )
        scr = wpool.tile([NROW, nband * WBAND], f32, tag="scr")
        nc.vector.scalar_tensor_tensor(
            scr[:].rearrange("p (b w) -> p b w", b=nband),
            cband[:].unsqueeze(2).to_broadcast((NROW, nband, WBAND)),
            -MARGIN,
            grid.rearrange("p (b w) -> p b w", b=nband),
            OP.add,
            OP.max,
            accum_out=accs[:, 0:1],
        )
        nc.sync.dma_start(d_out, accs[:])

    nc.compile()
    return nc


_PROGRAM_CACHE = {}


def _get_program(key):
    if key not in _PROGRAM_CACHE:
        _PROGRAM_CACHE[key] = _build_program(*key)
    return _PROGRAM_CACHE[key]


def _core_sel(a, p, n, core):
    R, H = core >> 1, core & 1
    sel = ((a & 3) == R) & ((n >> 8) == H)
    return R, H, a[sel] >> 2, p[sel], n[sel]


def _shard_inputs(samples, a, p, n, s_pad, nband, m_span):
    f8np = mybir.dt.np(mybir.dt.float8e4)
    in_maps = []
    nb_tot = []
    for core in range(NCORES):
        R, H, q, ps, ns = _core_sel(a, p, n, core)
        anchor_rows = np.arange(NROW, dtype=np.int64) * 4 + R
        others = np.setdiff1d(np.arange(N, dtype=np.int64), anchor_rows)
        perm = np.concatenate([anchor_rows, others])
        col_of = np.empty(N, dtype=np.int64)
        col_of[perm] = np.arange(N)

        idxs_all = np.full((NROW, N), -1, dtype=np.int16)
        idxs3 = np.full((NROW, s_pad * m_span), -1, dtype=np.int16)
        nb = 0
        order = np.argsort(q, kind="stable")
        qs, pss, nss = q[order], ps[order], ns[order]
        starts = np.searchsorted(qs, np.arange(NROW + 1))
        for qq in range(NROW):
            lo, hi = starts[qq], starts[qq + 1]
            if lo == hi:
                continue
            pq, nq = pss[lo:hi], nss[lo:hi]
            vals, inv = np.unique(pq, return_inverse=True)
            idxs_all[qq, col_of[vals]] = np.arange(len(vals), dtype=np.int16)
            band_start = 0
            for s in range(len(vals)):
                cols = col_of[nq[inv == s]]
                c = len(cols)
                nbd = -(-c // WBAND)
                idxs_all[qq, cols] = (
                    s_pad + band_start * WBAND + np.arange(c)
                ).astype(np.int16)
                idxs3[qq, s * m_span : s * m_span + nbd] = band_start + np.arange(
                    nbd, dtype=np.int16
                )
                band_start += nbd
            nb += band_start

        # st8[f, c] = samples[perm[c], f]; DR layout per group:
        # group g columns [256g, 256g+256) at [p, j*256 + c] = st8[j*128+p, c]
        st8 = np.ascontiguousarray(samples[perm].T).astype(f8np)  # [256, 512]
        st3 = st8.reshape(2, 128, N)  # [j, p, col]
        packa1 = np.ascontiguousarray(
            st3[:, :, 0:HC].transpose(1, 0, 2).reshape(128, 512)
        )
        grpB = st3[:, :, HC:N].transpose(1, 0, 2).reshape(128, 512)
        packl = st3[:, :, 0:NROW].transpose(1, 0, 2).reshape(128, 256)
        packa2 = np.ascontiguousarray(np.concatenate([grpB, packl], axis=1))
        packb = np.concatenate([idxs_all, idxs3], axis=1)
        in_maps.append({"packa1": packa1, "packa2": packa2, "packb": packb})
        nb_tot.append(nb)
    return in_maps, nb_tot


def kernel(samples, targets, anchor_idx, pos_idx, neg_idx, _want_trace=False):
    samples = np.asarray(samples, dtype=np.float32)
    a = np.asarray(anchor_idx).astype(np.int64)
    p = np.asarray(pos_idx).astype(np.int64)
    n = np.asarray(neg_idx).astype(np.int64)
    T = a.shape[0]
    assert samples.shape == (N, D)

    ok = (
        np.all((a >= 0) & (a < N) & (p >= 0) & (p < N) & (n >= 0) & (n < N))
        and len(np.unique(a * N + n)) == T
    )
    if not ok:
        raise NotImplementedError("inputs violate mined-triplet structure")

    # layout constants (max over cores)
    s_pad, nband, m_span = 2, 2, 1
    for core in range(NCORES):
        _, _, q, ps, ns = _core_sel(a, p, n, core)
        key = q * N + ps
        uniq, cnt = np.unique(key, return_counts=True)
        rows = uniq // N
        s_pad = max(s_pad, int(np.bincount(rows).max()))
        spans = -(-cnt // WBAND)
        m_span = max(m_span, int(spans.max()))
        nband = max(nband, int(np.bincount(rows, weights=spans).max()))
    s_pad += s_pad & 1
    nband += nband & 1
    if s_pad > 64 or nband * WBAND > 1500:
        raise NotImplementedError("palette/band layout too large")

    key = (s_pad, nband, m_span)
    nc = _get_program(key)
    in_maps, nb_tot = _shard_inputs(samples, a, p, n, s_pad, nband, m_span)
    res = run_bass_kernel_spmd(nc, in_maps, list(range(NCORES)), trace=_want_trace)
    total = 0.0
    for c in range(NCORES):
        o = res.results[c]["out"].astype(np.float64)
        total += float(o[:, 0].sum() - WBAND * o[:, 1].sum())
        total += WBAND * MARGIN * nb_tot[c]
    loss = np.float32(total / T)
    if _want_trace:
        return loss, res
    return loss


# revision 42
# speedup vs baseline: 1.0292x; 1.0292x over previous
"""Trainium2 Bass kernel for BatchWiseTripletDistanceLoss (banded scatter v8).

loss = mean_t relu(cos_d(s[a],s[p]) - cos_d(s[a],s[n]) + margin)
     = mean_t relu(sim[a,n] - sim[a,p] + margin)        (the "1-"s cancel)

Each of 8 cores owns 128 anchor rows (a mod 4) and half the negatives
(n >> 8).  On device it computes u = cos(anchor, all 512 samples) + 2 in
f16 (fp8 DoubleRow matmuls for dot products and squared norms, one
Abs_reciprocal_sqrt table op, an outer product of reciprocal norms), then
one gpsimd local_scatter distributes the u row into
  [palette | banded grid]:
  - palette slot s        <- u at the row's s-th distinct positive column
  - band cells            <- u at each triplet's negative column, with each
    row's triplets packed by positive-slot into width-32 bands (one slot
    per (row, band)).
A second tiny scatter builds cband[row, band] = u_pos of that band's slot.
One fused DVE op computes acc1 = sum max(cband - margin, grid) and another
acc2 = sum cband; the host combines
    sum relu = acc1 - 32*acc2 + 32*margin*n_used_bands
(empty cells/bands cancel exactly since u >= 1 > 0 and the margin is
applied in f32 inside the ALU, so no f16 grid-alignment bias).

The sample columns are processed in two groups (A = permuted cols 0..255,
which include all 128 anchors; B = cols 256..511) with separate DMAs, so
group A's norm/normalize chain overlaps group B's DMA + compute.  Host
work is layout / integer metadata only, plus the final tiny partial-sum
reduction.
"""
import sys

sys.path.insert(0, "/opt/trn_rl_repo")

from contextlib import ExitStack

import numpy as np

import concourse.bacc as bacc
import concourse.tile as tile
from concourse import mybir
from concourse.bass_utils import run_bass_kernel_spmd

DT = mybir.dt
OP = mybir.AluOpType
ACTF = mybir.ActivationFunctionType

N = 512
D = 256
MARGIN = 0.15
NCORES = 8
NROW = 128
WBAND = 32  # band width (cells per band)
HC = 256  # columns per group


def _build_program(s_pad: int, nband: int, m_span: int):
    """Build + compile the SPMD program (identical for all 8 cores)."""
    nc = bacc.Bacc(
        "TRN2", target_bir_lowering=False, debug=False, num_devices=NCORES
    )
    f32, i16, f16, f8 = DT.float32, DT.int16, DT.float16, DT.float8e4

    WIDE = s_pad + nband * WBAND
    NI3 = s_pad * m_span
    # group A: [j, c<256] DR layout (512 cols); group B: same for c>=256,
    # followed by the anchor lhsT block (256 cols).
    d_packa1 = nc.dram_tensor("packa1", [128, 512], f8, kind="ExternalInput").ap()
    d_packa2 = nc.dram_tensor("packa2", [128, 768], f8, kind="ExternalInput").ap()
    d_packb = nc.dram_tensor("packb", [NROW, N + NI3], i16, kind="ExternalInput").ap()
    d_out = nc.dram_tensor("out", [NROW, 2], f32, kind="ExternalOutput").ap()
    d_warm = nc.dram_tensor("warm", [1, 4], f16, kind="ExternalOutput").ap()

    with tile.TileContext(nc) as tc, ExitStack() as ctx:
        cpool = ctx.enter_context(tc.tile_pool(name="const", bufs=1))
        wpool = ctx.enter_context(tc.tile_pool(name="work", bufs=1))
        ppool = ctx.enter_context(tc.tile_pool(name="psum", bufs=1, space="PSUM"))

        # ---- input DMAs (three queues) ----------------------------------
        # descriptor writes spread across engines: scalar stays free for the
        # ACT table loads, which otherwise gate the first Square op
        packa1 = cpool.tile([128, 512], f8)
        nc.sync.dma_start(packa1[:], d_packa1)
        packa2 = cpool.tile([128, 768], f8)
        nc.gpsimd.dma_start(packa2[:], d_packa2)
        packb = cpool.tile([NROW, N + NI3], i16)
        nc.sync.dma_start(packb[:], d_packb)
        idxs_all = packb[:, 0:N]
        idxs3 = packb[:, N : N + NI3]
        packl = packa2[:, 512:768]

        # DR lhsT needs 16B-aligned j-step: ones read at [p, 0] and [p, 16]
        ones2 = cpool.tile([128, 32], f8)
        nc.vector.memset(ones2[:], 1.0)
        ones_lhsT = ones2[:].rearrange("p (j x) -> p j x", j=2)[:, :, 0:1]

        # ---- warmups hidden under the DMA phase -------------------------
        # rsqrt ACT table preload (the only ACT table this kernel uses)
        dumin = cpool.tile([1, 1], f32)
        nc.vector.memset(dumin[:], 4.0)
        dumout = cpool.tile([1, 1], f32)
        nc.scalar.activation(dumout[:], dumin[:], ACTF.Abs_reciprocal_sqrt)
        # gpsimd local_scatter ucode IRAM load (~6us, hidden here)
        dmy_d = cpool.tile([128, 2], f16)
        nc.vector.memset(dmy_d[:], 0.0)
        dmy_i = cpool.tile([128, 2], i16)
        nc.vector.memset(dmy_i[:], -1)
        dmy_o = cpool.tile([128, 2], f16)
        nc.gpsimd.local_scatter(
            dmy_o[:], dmy_d[:], dmy_i[:], channels=128, num_elems=2, num_idxs=2
        )

        DR = mybir.MatmulPerfMode.DoubleRow
        grpA = packa1[:].rearrange("p (j c) -> p j c", j=2)
        grpB = packa2[:, 0:512].rearrange("p (j c) -> p j c", j=2)
        lhsT = packl.rearrange("p (j q) -> p j q", j=2)

        u16 = wpool.tile([128, N], f16, tag="u16")
        simps, rbps = [], []
        for g, grp in enumerate((grpA, grpB)):
            # squares (ACT for A, DVE for B -> they run in parallel)
            sqg = wpool.tile([128, 512], f8, tag=f"sq{g}")
            if g == 0:
                nc.scalar.activation(sqg[:], packa1[:], ACTF.Square)
            else:
                nc.vector.tensor_tensor(
                    sqg[:], packa2[:, 0:512], packa2[:, 0:512], OP.mult
                )
            n2g = ppool.tile([1, HC], f32, tag=f"n2{g}")
            nc.tensor.matmul(
                n2g[:],
                ones_lhsT,
                sqg[:].rearrange("p (j c) -> p j c", j=2),
                start=True,
                stop=True,
                perf_mode=DR,
            )
            rrg = wpool.tile([1, HC], f16, tag=f"rr{g}")
            nc.scalar.activation(rrg[:], n2g[:], ACTF.Abs_reciprocal_sqrt)
            rbps.append(rrg)
            simpg = ppool.tile([128, HC], f32, tag=f"simp{g}")
            nc.tensor.matmul(simpg[:], lhsT, grp, start=True, stop=True, perf_mode=DR)
            simps.append(simpg)

        rrA = rbps[0]
        for g in range(2):
            # outer product of reciprocal norms (anchors all in group A)
            rbp2g = ppool.tile([128, HC], f32, tag=f"rbp2{g}")
            nc.tensor.matmul(
                rbp2g[:], rrA[:, 0:128], rbps[g][:], start=True, stop=True
            )
            simp_sbg = wpool.tile([128, HC], f32, tag=f"simp_sb{g}")
            nc.vector.tensor_scalar_add(simp_sbg[:], simps[g][:], 0.0)
            t1g = wpool.tile([128, HC], f32, tag=f"t1{g}")
            nc.vector.tensor_tensor(t1g[:], simp_sbg[:], rbp2g[:], OP.mult)
            nc.vector.tensor_scalar_add(u16[:, g * HC : (g + 1) * HC], t1g[:], 2.0)

        # ---- merged scatter: [palette | banded grid] --------------------
        dst = wpool.tile([NROW, WIDE], f16, tag="dst")
        nc.gpsimd.local_scatter(
            dst[:], u16[:], idxs_all, channels=128, num_elems=WIDE, num_idxs=N
        )
        palv = dst[:, 0:s_pad]
        grid = dst[:, s_pad:WIDE]
        # tiny write keyed on G1 keeps the output DMA queue awake so the
        # final result DMA doesn't pay the queue wake latency
        nc.sync.dma_start(d_warm, dst[0:1, 0:4])

        # crep = palette values replicated per span slot (exact f16 copy)
        crep = wpool.tile([NROW, NI3], f16, tag="crep")
        nc.vector.tensor_scalar_add(
            crep[:].rearrange("p (s j) -> p s j", s=s_pad),
            palv.unsqueeze(2).to_broadcast((NROW, s_pad, m_span)),
            0.0,
        )
        cband = wpool.tile([NROW, nband], f16, tag="cband")
        nc.gpsimd.local_scatter(
            cband[:], crep[:], idxs3, channels=128, num_elems=nband, num_idxs=NI3
        )

        # ---- fused evaluate + accumulate --------------------------------
        accs = wpool.tile([NROW, 2], f32, tag="accs")
        scr2 = wpool.tile([NROW, nband], f32, tag="scr2")
        nc.vector.tensor_scalar(
            scr2[:], cband[:], 0.0, 0.0, OP.add, OP.add, accum_out=accs[:, 1:2]
        )
        scr = wpool.tile([NROW, nband * WBAND], f32, tag="scr")
        nc.vector.scalar_tensor_tensor(
            scr[:].rearrange("p (b w) -> p b w", b=nband),
            cband[:].unsqueeze(2).to_broadcast((NROW, nband, WBAND)),
            -MARGIN,
            grid.rearrange("p (b w) -> p b w", b=nband),
            OP.add,
            OP.max,
            accum_out=accs[:, 0:1],
        )
        nc.sync.dma_start(d_out, accs[:])

    nc.compile()
    return nc


_PROGRAM_CACHE = {}


def _get_program(key):
    if key not in _PROGRAM_CACHE:
        _PROGRAM_CACHE[key] = _build_program(*key)
    return _PROGRAM_CACHE[key]


def _core_sel(a, p, n, core):
    R, H = core >> 1, core & 1
    sel = ((a & 3) == R) & ((n >> 8) == H)
    return R, H, a[sel] >> 2, p[sel], n[sel]


def _shard_inputs(samples, a, p, n, s_pad, nband, m_span):
    f8np = mybir.dt.np(mybir.dt.float8e4)
    in_maps = []
    nb_tot = []
    for core in range(NCORES):
        R, H, q, ps, ns = _core_sel(a, p, n, core)
        anchor_rows = np.arange(NROW, dtype=np.int64) * 4 + R
        others = np.setdiff1d(np.arange(N, dtype=np.int64), anchor_rows)
        perm = np.concatenate([anchor_rows, others])
        col_of = np.empty(N, dtype=np.int64)
        col_of[perm] = np.arange(N)

        idxs_all = np.full((NROW, N), -1, dtype=np.int16)
        idxs3 = np.full((NROW, s_pad * m_span), -1, dtype=np.int16)
        nb = 0
        order = np.argsort(q, kind="stable")
        qs, pss, nss = q[order], ps[order], ns[order]
        starts = np.searchsorted(qs, np.arange(NROW + 1))
        for qq in range(NROW):
            lo, hi = starts[qq], starts[qq + 1]
            if lo == hi:
                continue
            pq, nq = pss[lo:hi], nss[lo:hi]
            vals, inv = np.unique(pq, return_inverse=True)
            idxs_all[qq, col_of[vals]] = np.arange(len(vals), dtype=np.int16)
            band_start = 0
            for s in range(len(vals)):
                cols = col_of[nq[inv == s]]
                c = len(cols)
                nbd = -(-c // WBAND)
                idxs_all[qq, cols] = (
                    s_pad + band_start * WBAND + np.arange(c)
                ).astype(np.int16)
                idxs3[qq, s * m_span : s * m_span + nbd] = band_start + np.arange(
                    nbd, dtype=np.int16
                )
                band_start += nbd
            nb += band_start

        # st8[f, c] = samples[perm[c], f]; DR layout per group:
        # group g columns [256g, 256g+256) at [p, j*256 + c] = st8[j*128+p, c]
        st8 = np.ascontiguousarray(samples[perm].T).astype(f8np)  # [256, 512]
        st3 = st8.reshape(2, 128, N)  # [j, p, col]
        packa1 = np.ascontiguousarray(
            st3[:, :, 0:HC].transpose(1, 0, 2).reshape(128, 512)
        )
        grpB = st3[:, :, HC:N].transpose(1, 0, 2).reshape(128, 512)
        packl = st3[:, :, 0:NROW].transpose(1, 0, 2).reshape(128, 256)
        packa2 = np.ascontiguousarray(np.concatenate([grpB, packl], axis=1))
        packb = np.concatenate([idxs_all, idxs3], axis=1)
        in_maps.append({"packa1": packa1, "packa2": packa2, "packb": packb})
        nb_tot.append(nb)
    return in_maps, nb_tot


def kernel(samples, targets, anchor_idx, pos_idx, neg_idx, _want_trace=False):
    samples = np.asarray(samples, dtype=np.float32)
    a = np.asarray(anchor_idx).astype(np.int64)
    p = np.asarray(pos_idx).astype(np.int64)
    n = np.asarray(neg_idx).astype(np.int64)
    T = a.shape[0]
    assert samples.shape == (N, D)

    ok = (
        np.all((a >= 0) & (a < N) & (p >= 0) & (p < N) & (n >= 0) & (n < N))
        and len(np.unique(a * N + n)) == T
    )
    if not ok:
        raise NotImplementedError("inputs violate mined-triplet structure")

    # layout constants (max over cores)
    s_pad, nband, m_span = 2, 2, 1
    for core in range(NCORES):
        _, _, q, ps, ns = _core_sel(a, p, n, core)
        key = q * N + ps
        uniq, cnt = np.unique(key, return_counts=True)
        rows = uniq // N
        s_pad = max(s_pad, int(np.bincount(rows).max()))
        spans = -(-cnt // WBAND)
        m_span = max(m_span, int(spans.max()))
        nband = max(nband, int(np.bincount(rows, weights=spans).max()))
    s_pad += s_pad & 1
    nband += nband & 1
    if s_pad > 64 or nband * WBAND > 1500:
        raise NotImplementedError("palette/band layout too large")

    key = (s_pad, nband, m_span)
    nc = _get_program(key)
    in_maps, nb_tot = _shard_inputs(samples, a, p, n, s_pad, nband, m_span)
    res = run_bass_kernel_spmd(nc, in_maps, list(range(NCORES)), trace=_want_trace)
    total = 0.0
    for c in range(NCORES):
        o = res.results[c]["out"].astype(np.float64)
        total += float(o[:, 0].sum() - WBAND * o[:, 1].sum())
        total += WBAND * MARGIN * nb_tot[c]
    loss = np.float32(total / T)
    if _want_trace:
        return loss, res
    return loss


# revision 43
# speedup vs baseline: 1.0511x; 1.0213x over previous
"""Trainium2 Bass kernel for BatchWiseTripletDistanceLoss (banded scatter v11).

loss = mean_t relu(cos_d(s[a],s[p]) - cos_d(s[a],s[n]) + margin)
     = mean_t relu(sim[a,n] - sim[a,p] + margin)        (the "1-"s cancel)

Each of 8 cores owns 128 anchor rows (a mod 4) and half the negatives
(n >> 8).  On device it computes u = cos(anchor, all 512 samples) + 2 in
f16 (fp8 DoubleRow matmuls for dot products and squared norms, one
Abs_reciprocal_sqrt table op, an outer product of reciprocal norms), then
ONE gpsimd local_scatter distributes the u row into
  [cstart | banded grid]:
  - cstart[row, b]  <- u at a positive column, iff band b is the FIRST
    band of that positive's span
  - grid cells      <- u at each triplet's negative column, with each
    row's triplets packed by positive-slot into width-32 bands (one slot
    per (row, band)).
A single DVE prefix scan (state = maskz*state + cstart) forward-fills
cstart across each span, yielding cband[row, band] = u_pos of that band.
One fused DVE op computes acc1 = sum max(cband - margin, grid), another
acc2 = sum cband, and the host combines
    sum relu = acc1 - 32*acc2 + 32*margin*n_used_bands
(empty cells/bands cancel exactly since u >= 1 > 0, and the margin is
applied in f32 inside the ALU, so no f16 grid-alignment bias).

The sample columns are processed in two groups (A = permuted cols 0..255,
which include all 128 anchors; B = cols 256..511) with separate DMAs, so
group A's norm/normalize chain overlaps group B's DMA + compute.  Host
work is layout / integer metadata only, plus the final tiny partial-sum
reduction.
"""
import sys

sys.path.insert(0, "/opt/trn_rl_repo")

from contextlib import ExitStack

import numpy as np

import concourse.bacc as bacc
import concourse.tile as tile
from concourse import mybir
from concourse.bass_utils import run_bass_kernel_spmd

DT = mybir.dt
OP = mybir.AluOpType
ACTF = mybir.ActivationFunctionType

N = 512
D = 256
MARGIN = 0.15
NCORES = 8
NROW = 128
WBAND = 32  # band width (cells per band)
HC = 256  # columns per group


def _build_program(nband: int):
    """Build + compile the SPMD program (identical for all 8 cores)."""
    nc = bacc.Bacc(
        "TRN2", target_bir_lowering=False, debug=False, num_devices=NCORES
    )
    f32, i16, f16, f8 = DT.float32, DT.int16, DT.float16, DT.float8e4

    WIDE = nband + nband * WBAND
    # group A: [j, c<256] DR layout (512 cols); group B: same for c>=256,
    # followed by the anchor lhsT block (256 cols).
    d_packa1 = nc.dram_tensor("packa1", [128, 512], f8, kind="ExternalInput").ap()
    d_packa2 = nc.dram_tensor("packa2", [128, 768], f8, kind="ExternalInput").ap()
    d_packb = nc.dram_tensor("packb", [NROW, N + nband], i16, kind="ExternalInput").ap()
    d_out = nc.dram_tensor("out", [NROW, 2], f32, kind="ExternalOutput").ap()
    d_warm = nc.dram_tensor("warm", [1, 4], f16, kind="ExternalOutput").ap()

    with tile.TileContext(nc) as tc, ExitStack() as ctx:
        cpool = ctx.enter_context(tc.tile_pool(name="const", bufs=1))
        wpool = ctx.enter_context(tc.tile_pool(name="work", bufs=1))
        ppool = ctx.enter_context(tc.tile_pool(name="psum", bufs=1, space="PSUM"))

        # descriptor writes spread across engines: scalar stays free for the
        # ACT table loads, which otherwise gate the first Square op
        packa1 = cpool.tile([128, 512], f8)
        nc.sync.dma_start(packa1[:], d_packa1)
        packa2 = cpool.tile([128, 768], f8)
        nc.gpsimd.dma_start(packa2[:], d_packa2)
        packb = cpool.tile([NROW, N + nband], i16)
        nc.sync.dma_start(packb[:], d_packb)
        idxs_all = packb[:, 0:N]
        maskz = packb[:, N : N + nband].bitcast(f16)
        packl = packa2[:, 512:768]

        # DR lhsT needs 16B-aligned j-step: ones read at [p, 0] and [p, 16]
        ones2 = cpool.tile([128, 32], f8)
        nc.vector.memset(ones2[:], 1.0)
        ones_lhsT = ones2[:].rearrange("p (j x) -> p j x", j=2)[:, :, 0:1]

        # ---- warmups hidden under the DMA phase -------------------------
        # rsqrt ACT table preload (the only ACT table this kernel uses)
        dumin = cpool.tile([1, 1], f32)
        nc.vector.memset(dumin[:], 4.0)
        dumout = cpool.tile([1, 1], f32)
        nc.scalar.activation(dumout[:], dumin[:], ACTF.Abs_reciprocal_sqrt)
        # gpsimd local_scatter ucode IRAM load (~6us, hidden here)
        dmy_d = cpool.tile([128, 2], f16)
        nc.vector.memset(dmy_d[:], 0.0)
        dmy_i = cpool.tile([128, 2], i16)
        nc.vector.memset(dmy_i[:], -1)
        dmy_o = cpool.tile([128, 2], f16)
        nc.gpsimd.local_scatter(
            dmy_o[:], dmy_d[:], dmy_i[:], channels=128, num_elems=2, num_idxs=2
        )

        DR = mybir.MatmulPerfMode.DoubleRow
        grpA = packa1[:].rearrange("p (j c) -> p j c", j=2)
        grpB = packa2[:, 0:512].rearrange("p (j c) -> p j c", j=2)
        lhsT = packl.rearrange("p (j q) -> p j q", j=2)

        u16 = wpool.tile([128, N], f16, tag="u16")
        simps, rrs = [], []
        for g, grp in enumerate((grpA, grpB)):
            # squares (ACT for A, DVE for B -> they run in parallel)
            sqg = wpool.tile([128, 512], f8, tag=f"sq{g}")
            if g == 0:
                nc.scalar.activation(sqg[:], packa1[:], ACTF.Square)
            else:
                nc.vector.tensor_tensor(
                    sqg[:], packa2[:, 0:512], packa2[:, 0:512], OP.mult
                )
            n2g = ppool.tile([1, HC], f32, tag=f"n2{g}")
            nc.tensor.matmul(
                n2g[:],
                ones_lhsT,
                sqg[:].rearrange("p (j c) -> p j c", j=2),
                start=True,
                stop=True,
                perf_mode=DR,
            )
            rrg = wpool.tile([1, HC], f16, tag=f"rr{g}")
            nc.scalar.activation(rrg[:], n2g[:], ACTF.Abs_reciprocal_sqrt)
            rrs.append(rrg)
            simpg = ppool.tile([128, HC], f32, tag=f"simp{g}")
            nc.tensor.matmul(simpg[:], lhsT, grp, start=True, stop=True, perf_mode=DR)
            simps.append(simpg)

        rrA = rrs[0]
        for g in range(2):
            # outer product of reciprocal norms (anchors all in group A)
            rbp2g = ppool.tile([128, HC], f32, tag=f"rbp2{g}")
            nc.tensor.matmul(
                rbp2g[:], rrA[:, 0:128], rrs[g][:], start=True, stop=True
            )
            simp_sbg = wpool.tile([128, HC], f32, tag=f"simp_sb{g}")
            nc.vector.tensor_scalar_add(simp_sbg[:], simps[g][:], 0.0)
            t1g = wpool.tile([128, HC], f32, tag=f"t1{g}")
            nc.vector.tensor_tensor(t1g[:], simp_sbg[:], rbp2g[:], OP.mult)
            nc.vector.tensor_scalar_add(u16[:, g * HC : (g + 1) * HC], t1g[:], 2.0)

        # ---- merged scatter: [cstart | banded grid] ---------------------
        dst = wpool.tile([NROW, WIDE], f16, tag="dst")
        nc.gpsimd.local_scatter(
            dst[:], u16[:], idxs_all, channels=128, num_elems=WIDE, num_idxs=N
        )
        cstart = dst[:, 0:nband]
        grid = dst[:, nband:WIDE]
        # tiny write keyed on G1 keeps the output DMA queue awake so the
        # final result DMA doesn't pay the queue wake latency
        nc.sync.dma_start(d_warm, dst[0:1, 0:4])

        # forward-fill each span: state = maskz*state + cstart
        cband = wpool.tile([NROW, nband], f16, tag="cband")
        nc.vector.tensor_tensor_scan(
            cband[:], maskz, cstart, 0.0, OP.mult, OP.add
        )

        # ---- fused evaluate + accumulate --------------------------------
        accs = wpool.tile([NROW, 2], f32, tag="accs")
        scr2 = wpool.tile([NROW, nband], f32, tag="scr2")
        nc.vector.tensor_scalar(
            scr2[:], cband[:], 0.0, 0.0, OP.add, OP.add, accum_out=accs[:, 1:2]
        )
        scr = wpool.tile([NROW, nband * WBAND], f32, tag="scr")
        nc.vector.scalar_tensor_tensor(
            scr[:].rearrange("p (b w) -> p b w", b=nband),
            cband[:].unsqueeze(2).to_broadcast((NROW, nband, WBAND)),
            -MARGIN,
            grid.rearrange("p (b w) -> p b w", b=nband),
            OP.add,
            OP.max,
            accum_out=accs[:, 0:1],
        )
        nc.sync.dma_start(d_out, accs[:])

    nc.compile()
    return nc


_PROGRAM_CACHE = {}


def _get_program(key):
    if key not in _PROGRAM_CACHE:
        _PROGRAM_CACHE[key] = _build_program(*key)
    return _PROGRAM_CACHE[key]


def _core_sel(a, p, n, core):
    R, H = core >> 1, core & 1
    sel = ((a & 3) == R) & ((n >> 8) == H)
    return R, H, a[sel] >> 2, p[sel], n[sel]


def _shard_inputs(samples, a, p, n, nband):
    f8np = mybir.dt.np(mybir.dt.float8e4)
    one16 = np.float16(1.0).view(np.int16)
    in_maps = []
    nb_tot = []
    for core in range(NCORES):
        R, H, q, ps, ns = _core_sel(a, p, n, core)
        anchor_rows = np.arange(NROW, dtype=np.int64) * 4 + R
        others = np.setdiff1d(np.arange(N, dtype=np.int64), anchor_rows)
        perm = np.concatenate([anchor_rows, others])
        col_of = np.empty(N, dtype=np.int64)
        col_of[perm] = np.arange(N)

        idxs_all = np.full((NROW, N), -1, dtype=np.int16)
        maskz = np.zeros((NROW, nband), dtype=np.int16)
        nb = 0
        order = np.argsort(q, kind="stable")
        qs, pss, nss = q[order], ps[order], ns[order]
        starts = np.searchsorted(qs, np.arange(NROW + 1))
        for qq in range(NROW):
            lo, hi = starts[qq], starts[qq + 1]
            if lo == hi:
                continue
            pq, nq = pss[lo:hi], nss[lo:hi]
            vals, inv = np.unique(pq, return_inverse=True)
            band_start = 0
            for s in range(len(vals)):
                cols = col_of[nq[inv == s]]
                c = len(cols)
                nbd = -(-c // WBAND)
                # positive -> first band of its span; continuations get mask 1
                idxs_all[qq, col_of[vals[s]]] = band_start
                maskz[qq, band_start + 1 : band_start + nbd] = one16
                idxs_all[qq, cols] = (
                    nband + band_start * WBAND + np.arange(c)
                ).astype(np.int16)
                band_start += nbd
            nb += band_start

        # st8[f, c] = samples[perm[c], f]; DR layout per group:
        # group g columns [256g, 256g+256) at [p, j*256 + c] = st8[j*128+p, c]
        st8 = np.ascontiguousarray(samples[perm].T).astype(f8np)  # [256, 512]
        st3 = st8.reshape(2, 128, N)  # [j, p, col]
        packa1 = np.ascontiguousarray(
            st3[:, :, 0:HC].transpose(1, 0, 2).reshape(128, 512)
        )
        grpB = st3[:, :, HC:N].transpose(1, 0, 2).reshape(128, 512)
        packl = st3[:, :, 0:NROW].transpose(1, 0, 2).reshape(128, 256)
        packa2 = np.ascontiguousarray(np.concatenate([grpB, packl], axis=1))
        packb = np.concatenate([idxs_all, maskz], axis=1)
        in_maps.append({"packa1": packa1, "packa2": packa2, "packb": packb})
        nb_tot.append(nb)
    return in_maps, nb_tot


def kernel(samples, targets, anchor_idx, pos_idx, neg_idx, _want_trace=False):
    samples = np.asarray(samples, dtype=np.float32)
    a = np.asarray(anchor_idx).astype(np.int64)
    p = np.asarray(pos_idx).astype(np.int64)
    n = np.asarray(neg_idx).astype(np.int64)
    T = a.shape[0]
    assert samples.shape == (N, D)

    ok = (
        np.all((a >= 0) & (a < N) & (p >= 0) & (p < N) & (n >= 0) & (n < N))
        and len(np.unique(a * N + n)) == T
    )
    if not ok:
        raise NotImplementedError("inputs violate mined-triplet structure")

    # band-count layout constant (max over cores)
    nband = 2
    for core in range(NCORES):
        _, _, q, ps, ns = _core_sel(a, p, n, core)
        key = q * N + ps
        uniq, cnt = np.unique(key, return_counts=True)
        rows = uniq // N
        spans = -(-cnt // WBAND)
        nband = max(nband, int(np.bincount(rows, weights=spans).max()))
    nband += nband & 1
    if nband * WBAND > 1500:
        raise NotImplementedError("band layout too large")

    key = (nband,)
    nc = _get_program(key)
    in_maps, nb_tot = _shard_inputs(samples, a, p, n, nband)
    res = run_bass_kernel_spmd(nc, in_maps, list(range(NCORES)), trace=_want_trace)
    total = 0.0
    for c in range(NCORES):
        o = res.results[c]["out"].astype(np.float64)
        total += float(o[:, 0].sum() - WBAND * o[:, 1].sum())
        total += WBAND * MARGIN * nb_tot[c]
    loss = np.float32(total / T)
    if _want_trace:
        return loss, res
    return loss
